# revision 31
# baseline (speedup 1.0000x reference)
"""Viterbi decode kernel for TRN2 (Bass/Tile) — custom-DVE fused version.

Layout (per core, B_loc=16 batch rows):
  partition p = b*8 + ch   (b in [0,16), ch in [0,8));  cur = ch*16 + cl
  TRW  [128, 16, 128] f32 : TRW[b*8+ch, cl, q] = trans[q, ch*16+cl]
  POT  [128, T*16]   f32 : POT[b*8+ch, t*16+cl]  = pot[b, t, ch*16+cl]
  AHIST DRAM [128, T*16] : alpha_t[b, cur] = AHIST[b*8+ch, t*16+cl]

Forward per t: ONE fused custom-DVE op (VIT_SEGMAX: running max of
  (TRW + alpha_bcast) with a per-cl-page reset via a hand-built
  SUB_DIM_DONE step state) -> page maxes at scr[:, :, 127]; stage =
  m + pot; 8x stream_shuffle -> ALPHA_P broadcast; AHIST DMA per group.

Backward per t (descending): onehot(tag) via iota is_eq; fp32 PE
  transpose; ONE fp32 selector matmul (exact: 0/1 weights) -> W column;
  fused VIT_ROWMAX (cand = alpha + wcol, accum max) -> m*; fused
  VIT_FIRSTIDX (first p with cand==m*, ties lowest) -> tag.
"""
from contextlib import ExitStack

import numpy as np

import concourse.bass as bass
import concourse.tile as tile
from concourse import mybir
from concourse import dve_spec as DS
from concourse import dve_ops as DO
from concourse.dve_spec import (
    Spec, Src0, Src1, C0, C1, Zero, MaxNeg, AluOp, scan, select, eq, Idx,
)
from concourse.dve_uop import DveOpSpec, Trigger

F32 = mybir.dt.float32
ADD = mybir.AluOpType.add
MAX = mybir.AluOpType.max
IS_EQ = mybir.AluOpType.is_equal
NEG_BIG = float(np.float32(-3.0e38))


# ---------------------------------------------------------------- custom ops
def _lower_segmax(spec, ver):
    """lower() with a hand-built FSM: seed -> steady <-> (SUB_DIM_DONE) step,
    where the step state re-seeds the scan accumulator with the current
    element (MAX(MaxNeg, expr)) so the fold restarts at each page."""
    DS._validate_body(spec, ver)
    spec2 = DS._hoist_stream_invariant_ops(spec)
    scans = DS._collect(spec2.body, DS.Scan)
    latches = DS._collect(spec2.body, DS.Latch)
    assert len(scans) == 1 and not latches
    n_lanes, n_stages = DS.N_LANES[ver], DS.N_STAGES[ver]
    p = DS._build_placement(spec2, scans, n_stages, n_lanes)
    seed_ov, _ = DS._scan_overrides(scans, p.node_stage)
    sc = scans[0]
    d = p.node_stage[sc]
    step_ov = {d: DS._Stage(sc.op, MaxNeg, sc.expr)}
    body_lvs = DS._body_scan_leaves(spec2)
    consume = (Src0 in body_lvs, Src1 in body_lvs)
    states = [
        DS._State(placement=p, overrides=seed_ov, trigger=DS.COUNT_ONCE,
                  repeat=1, next=(1, 0, 0), write_out=False),
        DS._State(placement=p, consume=consume,
                  trigger=(Trigger.SRC_TENSOR_DONE, Trigger.SUB_DIM_DONE,
                           Trigger.NONE),
                  next=(0, 2, 0)),
        DS._State(placement=p, consume=consume, overrides=step_ov,
                  trigger=(Trigger.SRC_TENSOR_DONE, Trigger.SUB_DIM_DONE,
                           Trigger.COUNT),
                  next=(0, 2, 1), repeat=1),
    ]
    uops = [DS._assemble(s) for s in states]
    for u in uops:
        u.validate(ver)
    return uops


def _register(op, uops_by_ver=None):
    if any(o.name == op.name for o in DO.OPS):
        return
    DO.OPS.append(op)
    DO.CUSTOM_DVE_SPECS[op.name] = op.spec
    row = DO._CUSTOM_DVE_ROW_BASE + len(DO.OPS) - 1
    assert row < 0x20
    DO._SUB_OPCODE_FOR_NAME[op.name] = row
    if uops_by_ver:
        for ver, uops in uops_by_ver.items():
            DO._COMPILE_CACHE[(op.name, ver)] = DveOpSpec(
                name=op.name, opcode=row, uops=uops,
                rd1_en=DS._has_src1(op.spec))


def _sha_for(spec, ver):
    s = DveOpSpec(name="tmp", opcode=1, uops=DS.lower(spec, ver=ver),
                  rd1_en=DS._has_src1(spec))
    return s.sha(ver)


_OPS_CACHE = {}


def get_ops():
    if _OPS_CACHE:
        return _OPS_CACHE
    ver = "v3"

    segmax_spec = Spec(
        body=scan(AluOp.MAX, Src0 + Src1),
        reference=lambda in0, in1, s0, s1, imm2: np.maximum.accumulate(
            (in0.astype(np.float32) + in1), axis=-1))
    segmax = DO.DveOp("VIT_SEGMAX", segmax_spec, subdim=True, uops_sha={})
    _register(segmax, {ver: _lower_segmax(segmax_spec, ver)})

    def _ref_rowmax(in0, in1, c0, c1, c2):
        b = (in0.astype(np.float32) + in1).astype(np.float32)
        m = np.maximum(c0, b.reshape(b.shape[0], -1).max(axis=-1, keepdims=True))
        return b, m

    rowmax_spec = Spec(body=Src0 + Src1, accum=AluOp.MAX, accum_init=C0,
                       reference=_ref_rowmax)
    rowmax = DO.DveOp("VIT_ROWMAX", rowmax_spec, subdim=False,
                      uops_sha={ver: None})
    rowmax.uops_sha[ver] = _sha_for(rowmax_spec, ver)
    _register(rowmax)

    def _ref_firstidx(in0, in1, c0, c1, c2):
        P = in0.shape[0]
        x = in0.reshape(P, -1)
        idx = np.broadcast_to(np.arange(x.shape[1], dtype=np.float32), x.shape)
        masked = np.where(x == c0, idx, c1)
        return masked, np.minimum(c1, masked.min(axis=-1, keepdims=True))

    firstidx_spec = Spec(body=select(eq(Src0, C0), Idx + Zero, C1),
                         accum=AluOp.MIN, accum_init=C1,
                         reference=_ref_firstidx)
    firstidx = DO.DveOp("VIT_FIRSTIDX", firstidx_spec, subdim=False,
                        uops_sha={ver: None})
    firstidx.uops_sha[ver] = _sha_for(firstidx_spec, ver)
    _register(firstidx)

    _OPS_CACHE.update(segmax=segmax, rowmax=rowmax, firstidx=firstidx)
    return _OPS_CACHE


# ------------------------------------------------------------------ utility
def legalize_waits(nc):
    """This container's walrus accepts at most ONE sync wait per
    instruction; Tile emits drains/noops with many.  Split them into
    single-wait NoOps on the same engine."""
    n_split = 0
    for f in nc.m.functions:
        for blk in f.blocks:
            new = []
            for inst in blk.instructions:
                si = inst.sync_info
                if si is not None and si.on_wait and len(si.on_wait) > 1:
                    waits = list(si.on_wait)
                    for j, w in enumerate(waits[:-1]):
                        new.append(mybir.InstNoOp(
                            name=f"{inst.name}-sw{j}", engine=inst.engine,
                            sync_info=mybir.SyncInfo(on_wait=[w], on_update=[])))
                        n_split += 1
                    inst.sync_info = mybir.SyncInfo(
                        on_wait=[waits[-1]], on_update=list(si.on_update))
                new.append(inst)
            blk.instructions = new
    return n_split


def host_prep(inputs_np, trans_np, n_cores=8):
    """Full inputs -> per-core input maps (list of dicts)."""
    B, T, C = inputs_np.shape
    assert C == 128 and B % n_cores == 0
    bl = B // n_cores  # 16

    transT = np.ascontiguousarray(trans_np.T).astype(np.float32)  # [c, q]

    # Rotated TRW for the log-doubling alpha broadcast: the alpha tile for
    # partition p=b*8+ch holds, at slot s (cols s*16..s*16+16), the stage of
    # channel (ch+s)&7, i.e. alpha[b, ((ch+s)&7)*16+cl2].  Bake the matching
    # q-permutation into TRW so segmax still sums alpha[q]+trans[q,c]:
    #   TRW[b*8+ch, cl, s*16+cl2] = trans[((ch+s)&7)*16+cl2, ch*16+cl]
    trw1 = np.empty((8, 16, 8, 16), dtype=np.float32)  # [ch, cl, s, cl2]
    for ch in range(8):
        for s in range(8):
            src = ((ch + s) & 7) * 16
            # trans[src+cl2, ch*16+cl] -> [cl, cl2]
            trw1[ch, :, s, :] = trans_np[src:src + 16, ch * 16:ch * 16 + 16].T
    trw = np.tile(trw1.reshape(8, 16 * 128)[None], (bl, 1, 1))
    trw = np.ascontiguousarray(trw.reshape(128, 16 * 128), dtype=np.float32)

    iota128 = np.ascontiguousarray(
        np.tile(np.arange(128, dtype=np.float32)[None, :], (128, 1)))
    id128 = np.eye(128, dtype=np.float32)

    # Split transT into three bf16 terms whose fp32 PSUM accumulation
    # (round(round(hi+mid)+lo)) reconstructs transT BITWISE-exactly, so
    # the backward gather can use single-pass bf16 matmuls.  Verified
    # element-wise here; half-ulp ties are fixed by nudging lo.
    import ml_dtypes
    bf16 = ml_dtypes.bfloat16
    f32 = np.float32
    hi = transT.astype(bf16)
    r1 = (transT - hi.astype(f32)).astype(f32)
    mid = r1.astype(bf16)
    r2 = (r1 - mid.astype(f32)).astype(f32)
    lo = r2.astype(bf16)

    def _rec(lo_arr):
        return ((hi.astype(f32) + mid.astype(f32)).astype(f32)
                + lo_arr.astype(f32)).astype(f32)

    bad = _rec(lo) != transT
    if bad.any():
        cand_up = np.nextafter(lo.astype(f32), np.float32(np.inf)).astype(bf16)
        cand_dn = np.nextafter(lo.astype(f32), np.float32(-np.inf)).astype(bf16)
        for cand in (cand_up, cand_dn):
            fix = bad & (_rec(np.where(bad, cand, lo)) == transT)
            lo = np.where(fix, cand, lo).astype(bf16)
            bad = _rec(lo) != transT
    assert not bad.any(), "bf16 triple-split reconstruction not exact"

    in_maps = []
    for core in range(n_cores):
        pc = inputs_np[core * bl:(core + 1) * bl]  # [16, T, 128]
        pot = pc.reshape(bl, T, 8, 16).transpose(0, 2, 1, 3)
        pot = np.ascontiguousarray(pot.reshape(128, T * 16), dtype=np.float32)
        in_maps.append({
            "pot": pot, "trw": trw,
            "wth": hi, "wtm": mid, "wtl": lo,
            "iota128": iota128, "id128": id128,
        })
    return in_maps


def build(T=2048, UF=8, UB=8, legalize=True):
    """Build the Bass program. Returns nc."""
    OPS = get_ops()
    nc = bass.Bass()

    d_pot = nc.dram_tensor("pot", [128, T * 16], F32, kind="ExternalInput")
    d_trw = nc.dram_tensor("trw", [128, 16 * 128], F32, kind="ExternalInput")
    BF16D = mybir.dt.bfloat16
    d_wth = nc.dram_tensor("wth", [128, 128], BF16D, kind="ExternalInput")
    d_wtm = nc.dram_tensor("wtm", [128, 128], BF16D, kind="ExternalInput")
    d_wtl = nc.dram_tensor("wtl", [128, 128], BF16D, kind="ExternalInput")
    d_id128 = nc.dram_tensor("id128", [128, 128], F32, kind="ExternalInput")
    d_iota128 = nc.dram_tensor(
        "iota128", [128, 128], F32, kind="ExternalInput")
    KK = (T + 7) // 8 + 32
    d_tagsraw = nc.dram_tensor("tagsraw", [128, KK], F32,
                               kind="ExternalOutput")
    d_ahist = nc.dram_tensor("ahist", [128, T * 16], F32, kind="Internal")
    ahist_bt = d_ahist.rearrange("(b ch) (t cl) -> b t ch cl", ch=8, cl=16)

    with tile.TileContext(nc) as tc, ExitStack() as ctx:
        singles = ctx.enter_context(tc.tile_pool(name="singles", bufs=1))

        # ---------------- forward ----------------
        with (
            tc.tile_pool(name="potp", bufs=1) as potp,
            tc.tile_pool(name="stp", bufs=2) as stp,
            tc.tile_pool(name="scrp", bufs=2) as scrp,
        ):
            s_trw = singles.tile([128, 16, 128], F32)
            s_alpha = singles.tile([128, 128], F32)  # ALPHA_P
            nc.sync.dma_start(
                out=s_trw[:], in_=d_trw.rearrange("p (cl q) -> p cl q", cl=16))

            s_pot = potp.tile([128, T * 16], F32)
            NPC = 8  # split preload so early compute can start sooner
            for c in range(NPC):
                sl = slice(c * T * 16 // NPC, (c + 1) * T * 16 // NPC)
                nc.sync.dma_start(out=s_pot[:, sl], in_=d_pot[:, sl])

            # touch each preload chunk on DVE so the loop body carries no
            # extra DMA-queue waits (back-edge drain has limited wait slots)
            s_touch = singles.tile([128, NPC], F32)
            for c in range(NPC):
                nc.vector.tensor_copy(
                    s_touch[:, c:c + 1],
                    s_pot[:, c * T * 16 // NPC:c * T * 16 // NPC + 1])

            # alpha broadcast: slot s of partition (b,ch) holds the stage of
            # channel (ch+s)&7; TRW is pre-rotated to match.  The stage is
            # written straight into slot 0 by the producer, so only slots
            # 1-3 (independent -> pipeline) plus one rot-4 shuffle remain.
            def bcast_alpha():
                for s in (1, 2, 3):
                    mask = [(j & ~7) | (((j & 7) + s) & 7) for j in range(32)]
                    nc.vector.stream_shuffle(
                        s_alpha[:, s * 16:(s + 1) * 16], s_alpha[:, 0:16],
                        mask)
                mask4 = [(j & ~7) | (((j & 7) + 4) & 7) for j in range(32)]
                # split rot-4 so the first half only waits on slots 0-1 and
                # pipelines behind rot3 instead of serializing after it
                nc.vector.stream_shuffle(
                    s_alpha[:, 64:96], s_alpha[:, 0:32], mask4)
                nc.vector.stream_shuffle(
                    s_alpha[:, 96:128], s_alpha[:, 32:64], mask4)

            # t=0 init
            st0 = stp.tile([128, UF * 16], F32, tag="stage")
            nc.vector.tensor_copy(s_alpha[:, 0:16], s_pot[:, 0:16])
            nc.scalar.copy(st0[:, 0:16], s_alpha[:, 0:16])
            bcast_alpha()
            nc.sync.dma_start(out=d_ahist[:, 0:16], in_=st0[:, 0:16])

            def fwd_group(iv0, unroll):
                stage = stp.tile([128, UF * 16], F32, tag="stage")
                for k in range(unroll):
                    iv = iv0 + k * 16
                    scr = scrp.tile([128, 16, 128], F32, tag="scr")
                    alb = s_alpha[:]
                    al_bcast = bass.AP(
                        tensor=alb.tensor, offset=alb.offset,
                        ap=[list(alb.ap[0]), [0, 16], [1, 128]])
                    nc.vector._custom_dve(
                        OPS["segmax"], out=scr[:], in0=s_trw[:], in1=al_bcast)
                    # page maxes at scr[:, :, 127]
                    so = scr[:]
                    m_ap = bass.AP(tensor=so.tensor, offset=so.offset + 127,
                                   ap=[list(so.ap[0]), [128, 16]])
                    ksl = slice(k * 16, (k + 1) * 16)
                    # write the new stage straight into broadcast slot 0;
                    # the idle Scalar engine copies it to the AHIST staging
                    # tile off the DVE critical stream.
                    nc.vector.tensor_add(
                        s_alpha[:, 0:16], m_ap, s_pot[:, iv:iv + 16])
                    nc.scalar.copy(stage[:, ksl], s_alpha[:, 0:16])
                    bcast_alpha()
                nc.sync.dma_start(
                    out=d_ahist[:, iv0:iv0 + unroll * 16],
                    in_=stage[:, 0:unroll * 16])

            ngrp, rem = divmod(T - 1, UF)
            for g in range(ngrp):
                fwd_group(16 + g * UF * 16, UF)
            if rem:
                fwd_group(16 + ngrp * UF * 16, rem)

        # ---------------- backward: 128 parallel chains ----------------
        # Chain j = blk*16 + b backtraces time-block blk (length L=256)
        # plus a W=64 warmup into block blk+1 (greedy-started; coalescence
        # verified exactly on host, with numpy fallback).  All 128 chains
        # advance in lockstep: K=L+W steps instead of T serial steps.
        L, W = (T + 7) // 8, 32
        K = L + W
        t0s = [min(blk * L + L - 1 + W, T - 1) for blk in range(8)]
        with (
            tc.tile_pool(name="apg", bufs=3) as apgp,
            tc.tile_pool(name="pcand", bufs=2, space="PSUM") as pcand,
            tc.tile_pool(name="ptr", bufs=2, space="PSUM") as ptr,
        ):
            s_wth = singles.tile([128, 128], mybir.dt.bfloat16)
            nc.sync.dma_start(out=s_wth[:], in_=d_wth[:])
            s_wtm = singles.tile([128, 128], mybir.dt.bfloat16)
            nc.sync.dma_start(out=s_wtm[:], in_=d_wtm[:])
            s_wtl = singles.tile([128, 128], mybir.dt.bfloat16)
            nc.sync.dma_start(out=s_wtl[:], in_=d_wtl[:])
            s_id128 = singles.tile([128, 128], F32)
            nc.sync.dma_start(out=s_id128[:], in_=d_id128[:])
            s_iota128 = singles.tile([128, 128], F32)
            nc.sync.dma_start(out=s_iota128[:], in_=d_iota128[:])

            BF16 = mybir.dt.bfloat16
            s_tags = singles.tile([128, K], F32)
            s_tag = singles.tile([128, 1], F32)
            s_mv = singles.tile([128, 1], F32)
            # one-hot + identity are exact in bf16 -> single-pass transpose
            s_oh = singles.tile([128, 128], BF16)
            s_idb = singles.tile([128, 128], BF16)
            nc.vector.tensor_copy(s_idb[:], s_id128[:])
            s_ohT = singles.tile([128, 128], BF16)
            s_cand = singles.tile([128, 128], F32)
            s_junk = singles.tile([128, 128], F32)
            s_zero = singles.tile([128, 128], F32)
            nc.vector.memset(s_zero[:], 0)

            def tail(k):
                # from cand stream in s_cand + row max in s_mv: pick first
                # argmax as tag, record it, and build one-hot^T for the next
                # gather matmul.
                nc.vector._custom_dve(
                    OPS["firstidx"], out=s_junk[:], in0=s_cand[:],
                    s0=s_mv[:], s1=1.0e9, accum_out=s_tag[:])
                nc.scalar.copy(s_tags[:, k:k + 1], s_tag[:])
                nc.vector.tensor_scalar(
                    out=s_oh[:], in0=s_iota128[:], scalar1=s_tag[:],
                    scalar2=None, op0=IS_EQ)
                pT = ptr.tile([128, 128], BF16, tag="pT")
                nc.tensor.transpose(pT[:], s_oh[:], s_idb[:])
                nc.vector.tensor_copy(s_ohT[:], pT[:])

            # k=0: greedy init tag[j] = argmax_c alpha_{t0(blk)}[b, c]
            s_a0 = singles.tile([128, 128], F32)
            for blk in range(8):
                nc.sync.dma_start(
                    out=s_a0[blk * 16:(blk + 1) * 16, :],
                    in_=ahist_bt[:, t0s[blk]:t0s[blk] + 1, :, :])
            nc.vector._custom_dve(
                OPS["rowmax"], out=s_cand[:], in0=s_a0[:], in1=s_zero[:],
                s0=NEG_BIG, accum_out=s_mv[:])
            tail(0)

            # lockstep groups of 8: prefetch alpha rows for all 8 blocks.
            # apg layout [j, ch, i, cl]: the (i, cl) block of one (b, ch)
            # DRAM row is contiguous, keeping both DMA APs at 3 dims.
            av = d_ahist.rearrange("(b ch) f -> b ch f", ch=8)
            ngr = (K - 1 + 7) // 8
            for g in range(ngr):
                klo, khi = 8 * g + 1, min(8 * g + 8, K - 1)
                apg = apgp.tile([128, 8, 8, 16], F32, tag="apg")
                for blk in range(8):
                    lo = t0s[blk] - (8 * g + 8)
                    ioff = max(0, -lo)
                    nc.sync.dma_start(
                        out=apg[blk * 16:(blk + 1) * 16]
                        .rearrange("p ch t cl -> p ch (t cl)")
                        [:, :, ioff * 16:128],
                        in_=av[:, :, (lo + ioff) * 16:(lo + 8) * 16])
                for k in range(klo, khi + 1):
                    i = 8 * (g + 1) - k
                    cand = pcand.tile([128, 128], F32, tag="cand")
                    # cand_psum[j, p] = trans[p, tag_j], reconstructed
                    # bitwise-exactly from three single-pass bf16 matmuls
                    # sharing the one-hot weights; the alpha term is added
                    # by rowmax itself (Src0 + Src1).
                    nc.tensor.matmul(cand[:], s_ohT[:], s_wth[:],
                                     start=True, stop=False)
                    nc.tensor.matmul(cand[:], s_ohT[:], s_wtm[:],
                                     start=False, stop=False)
                    nc.tensor.matmul(cand[:], s_ohT[:], s_wtl[:],
                                     start=False, stop=True)
                    nc.vector._custom_dve(
                        OPS["rowmax"],
                        out=s_cand[:].rearrange("p (ch cl) -> p ch cl", ch=8),
                        in0=apg[:, :, i, :],
                        in1=cand[:].rearrange("p (ch cl) -> p ch cl", ch=8),
                        s0=NEG_BIG, accum_out=s_mv[:])
                    if k < K - 1:
                        tail(k)
                    else:
                        nc.vector._custom_dve(
                            OPS["firstidx"], out=s_junk[:], in0=s_cand[:],
                            s0=s_mv[:], s1=1.0e9, accum_out=s_tag[:])
                        nc.scalar.copy(s_tags[:, k:k + 1], s_tag[:])

            nc.sync.dma_start(out=d_tagsraw[:], in_=s_tags[:])

    mybir.codegen_inst_isa_subclasses(nc)
    if legalize:
        legalize_waits(nc)
    return nc


_NC_CACHE = {}


def _get_nc(T):
    if T not in _NC_CACHE:
        _NC_CACHE[T] = build(T=T, UF=8, UB=8)
    return _NC_CACHE[T]


def _viterbi_row_np(pot_b, trans):
    """Exact numpy Viterbi decode for one batch row (fallback path)."""
    T, C = pot_b.shape
    alphas = np.empty((T, C), dtype=np.float32)
    alphas[0] = pot_b[0]
    for t in range(1, T):
        scores = alphas[t - 1][:, None] + trans
        alphas[t] = scores.max(axis=0).astype(np.float32) + pot_b[t]
    tags = np.empty(T, dtype=np.int64)
    tags[T - 1] = int(alphas[T - 1].argmax())
    for t in range(T - 1, 0, -1):
        cand = alphas[t - 1] + trans[:, tags[t]]
        tags[t - 1] = int(cand.argmax())
    return tags


def _assemble_core(raw, pot_core, trans, T):
    """raw [128, K] f32 -> tags [16, T] with exact coalescence check.

    Chain j = blk*16+b covers t in [blk*L, (blk+1)*L); its column k holds
    the tag at time t0(blk)-k.  Warmup columns k<W overlap block blk+1's
    coverage; if any position matches, the whole chain is exact."""
    L = (T + 7) // 8
    Wp = 32
    raw = raw.astype(np.int64)
    t0s = [min(blk * L + L - 1 + Wp, T - 1) for blk in range(8)]
    tags = np.empty((16, T), dtype=np.int64)
    for b in range(16):
        ok = True
        for blk in range(7, -1, -1):
            j = blk * 16 + b
            t0 = t0s[blk]
            ks = t0 - (blk * L + L - 1)          # first coverage column
            cov = raw[j, ks:ks + L][::-1]        # tags at t = blk*L .. +L-1
            if blk < 7:
                # warmup cols k in [0, ks): t = t0-k inside blk+1 coverage
                wt_ = raw[j, 0:ks][::-1]         # t = (blk+1)*L .. t0
                truth = tags[b, (blk + 1) * L:(blk + 1) * L + ks]
                if not np.any(wt_ == truth):
                    ok = False
            tags[b, blk * L:(blk + 1) * L] = cov
        if not ok:
            tags[b] = _viterbi_row_np(pot_core[b], trans)
    return tags


def kernel(inputs, transitions):
    """Full-input Viterbi decode on 8 NeuronCores (data-parallel over batch)."""
    from concourse import bass_utils

    inputs = np.asarray(inputs)
    transitions = np.asarray(transitions)
    B, T, C = inputs.shape
    n_cores = 8
    in_maps = host_prep(inputs, transitions, n_cores=n_cores)
    nc = _get_nc(T)
    res = bass_utils.run_bass_kernel_spmd(
        nc, in_maps, core_ids=list(range(n_cores)))
    bl = B // n_cores
    parts = []
    for core in range(n_cores):
        raw = res.results[core]["tagsraw"]
        pot_core = inputs[core * bl:(core + 1) * bl]
        parts.append(_assemble_core(raw, pot_core, transitions, T))
    tags = np.concatenate(parts, axis=0)  # [B, T]
    return tags.astype(inputs.dtype)



# revision 32
# speedup vs baseline: 1.1711x; 1.1711x over previous
"""Viterbi decode kernel for TRN2 (Bass/Tile) — custom-DVE fused version.

Layout (per core, B_loc=16 batch rows):
  partition p = b*8 + ch   (b in [0,16), ch in [0,8));  cur = ch*16 + cl
  TRW  [128, 16, 128] f32 : TRW[b*8+ch, cl, q] = trans[q, ch*16+cl]
  POT  [128, T*16]   f32 : POT[b*8+ch, t*16+cl]  = pot[b, t, ch*16+cl]
  AHIST DRAM [128, T*16] : alpha_t[b, cur] = AHIST[b*8+ch, t*16+cl]

Forward per t: ONE fused custom-DVE op (VIT_SEGMAX: running max of
  (TRW + alpha_bcast) with a per-cl-page reset via a hand-built
  SUB_DIM_DONE step state) -> page maxes at scr[:, :, 127]; stage =
  m + pot; 8x stream_shuffle -> ALPHA_P broadcast; AHIST DMA per group.

Backward per t (descending): onehot(tag) via iota is_eq; fp32 PE
  transpose; ONE fp32 selector matmul (exact: 0/1 weights) -> W column;
  fused VIT_ROWMAX (cand = alpha + wcol, accum max) -> m*; fused
  VIT_FIRSTIDX (first p with cand==m*, ties lowest) -> tag.
"""
from contextlib import ExitStack

import numpy as np

import concourse.bass as bass
import concourse.tile as tile
from concourse import mybir
from concourse import dve_spec as DS
from concourse import dve_ops as DO
from concourse.dve_spec import (
    Spec, Src0, Src1, C0, C1, Zero, MaxNeg, AluOp, scan, select, eq, Idx,
)
from concourse.dve_uop import DveOpSpec, Trigger

F32 = mybir.dt.float32
ADD = mybir.AluOpType.add
MAX = mybir.AluOpType.max
IS_EQ = mybir.AluOpType.is_equal
NEG_BIG = float(np.float32(-3.0e38))


# ---------------------------------------------------------------- custom ops
def _lower_segmax(spec, ver):
    """lower() with a hand-built FSM: seed -> steady <-> (SUB_DIM_DONE) step,
    where the step state re-seeds the scan accumulator with the current
    element (MAX(MaxNeg, expr)) so the fold restarts at each page."""
    DS._validate_body(spec, ver)
    spec2 = DS._hoist_stream_invariant_ops(spec)
    scans = DS._collect(spec2.body, DS.Scan)
    latches = DS._collect(spec2.body, DS.Latch)
    assert len(scans) == 1 and not latches
    n_lanes, n_stages = DS.N_LANES[ver], DS.N_STAGES[ver]
    p = DS._build_placement(spec2, scans, n_stages, n_lanes)
    seed_ov, _ = DS._scan_overrides(scans, p.node_stage)
    sc = scans[0]
    d = p.node_stage[sc]
    step_ov = {d: DS._Stage(sc.op, MaxNeg, sc.expr)}
    body_lvs = DS._body_scan_leaves(spec2)
    consume = (Src0 in body_lvs, Src1 in body_lvs)
    states = [
        DS._State(placement=p, overrides=seed_ov, trigger=DS.COUNT_ONCE,
                  repeat=1, next=(1, 0, 0), write_out=False),
        DS._State(placement=p, consume=consume,
                  trigger=(Trigger.SRC_TENSOR_DONE, Trigger.SUB_DIM_DONE,
                           Trigger.NONE),
                  next=(0, 2, 0)),
        DS._State(placement=p, consume=consume, overrides=step_ov,
                  trigger=(Trigger.SRC_TENSOR_DONE, Trigger.SUB_DIM_DONE,
                           Trigger.COUNT),
                  next=(0, 2, 1), repeat=1),
    ]
    uops = [DS._assemble(s) for s in states]
    for u in uops:
        u.validate(ver)
    return uops


def _register(op, uops_by_ver=None):
    if any(o.name == op.name for o in DO.OPS):
        return
    DO.OPS.append(op)
    DO.CUSTOM_DVE_SPECS[op.name] = op.spec
    row = DO._CUSTOM_DVE_ROW_BASE + len(DO.OPS) - 1
    assert row < 0x20
    DO._SUB_OPCODE_FOR_NAME[op.name] = row
    if uops_by_ver:
        for ver, uops in uops_by_ver.items():
            DO._COMPILE_CACHE[(op.name, ver)] = DveOpSpec(
                name=op.name, opcode=row, uops=uops,
                rd1_en=DS._has_src1(op.spec))


def _sha_for(spec, ver):
    s = DveOpSpec(name="tmp", opcode=1, uops=DS.lower(spec, ver=ver),
                  rd1_en=DS._has_src1(spec))
    return s.sha(ver)


_OPS_CACHE = {}


def get_ops():
    if _OPS_CACHE:
        return _OPS_CACHE
    ver = "v3"

    segmax_spec = Spec(
        body=scan(AluOp.MAX, Src0 + Src1),
        reference=lambda in0, in1, s0, s1, imm2: np.maximum.accumulate(
            (in0.astype(np.float32) + in1), axis=-1))
    segmax = DO.DveOp("VIT_SEGMAX", segmax_spec, subdim=True, uops_sha={})
    _register(segmax, {ver: _lower_segmax(segmax_spec, ver)})

    def _ref_rowmax(in0, in1, c0, c1, c2):
        b = (in0.astype(np.float32) + in1).astype(np.float32)
        m = np.maximum(c0, b.reshape(b.shape[0], -1).max(axis=-1, keepdims=True))
        return b, m

    rowmax_spec = Spec(body=Src0 + Src1, accum=AluOp.MAX, accum_init=C0,
                       reference=_ref_rowmax)
    rowmax = DO.DveOp("VIT_ROWMAX", rowmax_spec, subdim=False,
                      uops_sha={ver: None})
    rowmax.uops_sha[ver] = _sha_for(rowmax_spec, ver)
    _register(rowmax)

    def _ref_firstidx(in0, in1, c0, c1, c2):
        P = in0.shape[0]
        x = in0.reshape(P, -1)
        idx = np.broadcast_to(np.arange(x.shape[1], dtype=np.float32), x.shape)
        masked = np.where(x == c0, idx, c1)
        return masked, np.minimum(c1, masked.min(axis=-1, keepdims=True))

    firstidx_spec = Spec(body=select(eq(Src0, C0), Idx + Zero, C1),
                         accum=AluOp.MIN, accum_init=C1,
                         reference=_ref_firstidx)
    firstidx = DO.DveOp("VIT_FIRSTIDX", firstidx_spec, subdim=False,
                        uops_sha={ver: None})
    firstidx.uops_sha[ver] = _sha_for(firstidx_spec, ver)
    _register(firstidx)

    _OPS_CACHE.update(segmax=segmax, rowmax=rowmax, firstidx=firstidx)
    return _OPS_CACHE


# ------------------------------------------------------------------ utility
def legalize_waits(nc):
    """This container's walrus accepts at most ONE sync wait per
    instruction; Tile emits drains/noops with many.  Split them into
    single-wait NoOps on the same engine."""
    n_split = 0
    for f in nc.m.functions:
        for blk in f.blocks:
            new = []
            for inst in blk.instructions:
                si = inst.sync_info
                if si is not None and si.on_wait and len(si.on_wait) > 1:
                    waits = list(si.on_wait)
                    for j, w in enumerate(waits[:-1]):
                        new.append(mybir.InstNoOp(
                            name=f"{inst.name}-sw{j}", engine=inst.engine,
                            sync_info=mybir.SyncInfo(on_wait=[w], on_update=[])))
                        n_split += 1
                    inst.sync_info = mybir.SyncInfo(
                        on_wait=[waits[-1]], on_update=list(si.on_update))
                new.append(inst)
            blk.instructions = new
    return n_split


def host_prep(inputs_np, trans_np, n_cores=8):
    """Full inputs -> per-core input maps (list of dicts)."""
    B, T, C = inputs_np.shape
    assert C == 128 and B % n_cores == 0
    bl = B // n_cores  # 16

    transT = np.ascontiguousarray(trans_np.T).astype(np.float32)  # [c, q]

    # Rotated TRW for the log-doubling alpha broadcast: the alpha tile for
    # partition p=b*8+ch holds, at slot s (cols s*16..s*16+16), the stage of
    # channel (ch+s)&7, i.e. alpha[b, ((ch+s)&7)*16+cl2].  Bake the matching
    # q-permutation into TRW so segmax still sums alpha[q]+trans[q,c]:
    #   TRW[b*8+ch, cl, s*16+cl2] = trans[((ch+s)&7)*16+cl2, ch*16+cl]
    trw1 = np.empty((8, 16, 8, 16), dtype=np.float32)  # [ch, cl, s, cl2]
    for ch in range(8):
        for s in range(8):
            src = ((ch + s) & 7) * 16
            # trans[src+cl2, ch*16+cl] -> [cl, cl2]
            trw1[ch, :, s, :] = trans_np[src:src + 16, ch * 16:ch * 16 + 16].T
    trw = np.tile(trw1.reshape(8, 16 * 128)[None], (bl, 1, 1))
    trw = np.ascontiguousarray(trw.reshape(128, 16 * 128), dtype=np.float32)

    iota128 = np.ascontiguousarray(
        np.tile(np.arange(128, dtype=np.float32)[None, :], (128, 1)))
    id128 = np.eye(128, dtype=np.float32)

    in_maps = []
    for core in range(n_cores):
        pc = inputs_np[core * bl:(core + 1) * bl]  # [16, T, 128]
        pot = pc.reshape(bl, T, 8, 16).transpose(0, 2, 1, 3)
        pot = np.ascontiguousarray(pot.reshape(128, T * 16), dtype=np.float32)
        in_maps.append({
            "pot": pot, "trw": trw, "wt": transT,
            "iota128": iota128, "id128": id128,
        })
    return in_maps


def build(T=2048, UF=8, UB=8, legalize=True):
    """Build the Bass program. Returns nc."""
    OPS = get_ops()
    nc = bass.Bass()

    d_pot = nc.dram_tensor("pot", [128, T * 16], F32, kind="ExternalInput")
    d_trw = nc.dram_tensor("trw", [128, 16 * 128], F32, kind="ExternalInput")
    d_wt = nc.dram_tensor("wt", [128, 128], F32, kind="ExternalInput")
    d_id128 = nc.dram_tensor("id128", [128, 128], F32, kind="ExternalInput")
    d_iota128 = nc.dram_tensor(
        "iota128", [128, 128], F32, kind="ExternalInput")
    KK = (T + 7) // 8 + 32
    d_tagsraw = nc.dram_tensor("tagsraw", [128, KK], F32,
                               kind="ExternalOutput")
    d_ahist = nc.dram_tensor("ahist", [128, T * 16], F32, kind="Internal")
    ahist_bt = d_ahist.rearrange("(b ch) (t cl) -> b t ch cl", ch=8, cl=16)

    with tile.TileContext(nc) as tc, ExitStack() as ctx:
        singles = ctx.enter_context(tc.tile_pool(name="singles", bufs=1))

        # ---------------- forward ----------------
        with (
            tc.tile_pool(name="potp", bufs=1) as potp,
            tc.tile_pool(name="stp", bufs=2) as stp,
            tc.tile_pool(name="scrp", bufs=2) as scrp,
        ):
            s_trw = singles.tile([128, 16, 128], F32)
            s_alpha = singles.tile([128, 128], F32)  # ALPHA_P
            nc.sync.dma_start(
                out=s_trw[:], in_=d_trw.rearrange("p (cl q) -> p cl q", cl=16))

            s_pot = potp.tile([128, T * 16], F32)
            NPC = 8  # split preload so early compute can start sooner
            for c in range(NPC):
                sl = slice(c * T * 16 // NPC, (c + 1) * T * 16 // NPC)
                nc.sync.dma_start(out=s_pot[:, sl], in_=d_pot[:, sl])

            # touch each preload chunk on DVE so the loop body carries no
            # extra DMA-queue waits (back-edge drain has limited wait slots)
            s_touch = singles.tile([128, NPC], F32)
            for c in range(NPC):
                nc.vector.tensor_copy(
                    s_touch[:, c:c + 1],
                    s_pot[:, c * T * 16 // NPC:c * T * 16 // NPC + 1])

            # alpha broadcast: slot s of partition (b,ch) holds the stage of
            # channel (ch+s)&7; TRW is pre-rotated to match.  The stage is
            # written straight into slot 0 by the producer, so only slots
            # 1-3 (independent -> pipeline) plus one rot-4 shuffle remain.
            def bcast_alpha():
                for s in (1, 2, 3):
                    mask = [(j & ~7) | (((j & 7) + s) & 7) for j in range(32)]
                    nc.vector.stream_shuffle(
                        s_alpha[:, s * 16:(s + 1) * 16], s_alpha[:, 0:16],
                        mask)
                mask4 = [(j & ~7) | (((j & 7) + 4) & 7) for j in range(32)]
                nc.vector.stream_shuffle(
                    s_alpha[:, 64:128], s_alpha[:, 0:64], mask4)

            # t=0 init
            st0 = stp.tile([128, UF * 16], F32, tag="stage")
            nc.vector.tensor_copy(s_alpha[:, 0:16], s_pot[:, 0:16])
            nc.scalar.copy(st0[:, 0:16], s_alpha[:, 0:16])
            bcast_alpha()
            nc.sync.dma_start(out=d_ahist[:, 0:16], in_=st0[:, 0:16])

            def fwd_group(iv0, unroll):
                stage = stp.tile([128, UF * 16], F32, tag="stage")
                for k in range(unroll):
                    iv = iv0 + k * 16
                    scr = scrp.tile([128, 16, 128], F32, tag="scr")
                    alb = s_alpha[:]
                    al_bcast = bass.AP(
                        tensor=alb.tensor, offset=alb.offset,
                        ap=[list(alb.ap[0]), [0, 16], [1, 128]])
                    nc.vector._custom_dve(
                        OPS["segmax"], out=scr[:], in0=s_trw[:], in1=al_bcast)
                    # page maxes at scr[:, :, 127]
                    so = scr[:]
                    m_ap = bass.AP(tensor=so.tensor, offset=so.offset + 127,
                                   ap=[list(so.ap[0]), [128, 16]])
                    ksl = slice(k * 16, (k + 1) * 16)
                    # write the new stage straight into broadcast slot 0;
                    # the idle Scalar engine copies it to the AHIST staging
                    # tile off the DVE critical stream.
                    nc.vector.tensor_add(
                        s_alpha[:, 0:16], m_ap, s_pot[:, iv:iv + 16])
                    nc.scalar.copy(stage[:, ksl], s_alpha[:, 0:16])
                    bcast_alpha()
                nc.sync.dma_start(
                    out=d_ahist[:, iv0:iv0 + unroll * 16],
                    in_=stage[:, 0:unroll * 16])

            ngrp, rem = divmod(T - 1, UF)
            for g in range(ngrp):
                fwd_group(16 + g * UF * 16, UF)
            if rem:
                fwd_group(16 + ngrp * UF * 16, rem)

        # ---------------- backward: 128 parallel chains ----------------
        # Chain j = blk*16 + b backtraces time-block blk (length L=256)
        # plus a W=64 warmup into block blk+1 (greedy-started; coalescence
        # verified exactly on host, with numpy fallback).  All 128 chains
        # advance in lockstep: K=L+W steps instead of T serial steps.
        L, W = (T + 7) // 8, 32
        K = L + W
        t0s = [min(blk * L + L - 1 + W, T - 1) for blk in range(8)]
        with (
            tc.tile_pool(name="apg", bufs=3) as apgp,
            tc.tile_pool(name="pcand", bufs=2, space="PSUM") as pcand,
            tc.tile_pool(name="ptr", bufs=2, space="PSUM") as ptr,
        ):
            s_wt = singles.tile([128, 128], F32)
            nc.sync.dma_start(out=s_wt[:], in_=d_wt[:])
            s_id128 = singles.tile([128, 128], F32)
            nc.sync.dma_start(out=s_id128[:], in_=d_id128[:])
            s_iota128 = singles.tile([128, 128], F32)
            nc.sync.dma_start(out=s_iota128[:], in_=d_iota128[:])

            BF16 = mybir.dt.bfloat16
            s_tags = singles.tile([128, K], F32)
            s_tag = singles.tile([128, 1], F32)
            s_mv = singles.tile([128, 1], F32)
            # one-hot + identity are exact in bf16 -> single-pass transpose
            s_oh = singles.tile([128, 128], BF16)
            s_idb = singles.tile([128, 128], BF16)
            nc.vector.tensor_copy(s_idb[:], s_id128[:])
            s_ohT = singles.tile([128, 128], F32)
            s_cand = singles.tile([128, 128], F32)
            s_junk = singles.tile([128, 128], F32)
            s_zero = singles.tile([128, 128], F32)
            nc.vector.memset(s_zero[:], 0)

            def tail(k):
                # from cand stream in s_cand + row max in s_mv: pick first
                # argmax as tag, record it, and build one-hot^T for the next
                # gather matmul.
                nc.vector._custom_dve(
                    OPS["firstidx"], out=s_junk[:], in0=s_cand[:],
                    s0=s_mv[:], s1=1.0e9, accum_out=s_tag[:])
                nc.scalar.copy(s_tags[:, k:k + 1], s_tag[:])
                nc.vector.tensor_scalar(
                    out=s_oh[:], in0=s_iota128[:], scalar1=s_tag[:],
                    scalar2=None, op0=IS_EQ)
                pT = ptr.tile([128, 128], BF16, tag="pT")
                nc.tensor.transpose(pT[:], s_oh[:], s_idb[:])
                nc.vector.tensor_copy(s_ohT[:], pT[:])

            # k=0: greedy init tag[j] = argmax_c alpha_{t0(blk)}[b, c]
            s_a0 = singles.tile([128, 128], F32)
            for blk in range(8):
                nc.sync.dma_start(
                    out=s_a0[blk * 16:(blk + 1) * 16, :],
                    in_=ahist_bt[:, t0s[blk]:t0s[blk] + 1, :, :])
            nc.vector._custom_dve(
                OPS["rowmax"], out=s_cand[:], in0=s_a0[:], in1=s_zero[:],
                s0=NEG_BIG, accum_out=s_mv[:])
            tail(0)

            # lockstep groups of 8: prefetch alpha rows for all 8 blocks.
            # apg layout [j, ch, i, cl]: the (i, cl) block of one (b, ch)
            # DRAM row is contiguous, keeping both DMA APs at 3 dims.
            av = d_ahist.rearrange("(b ch) f -> b ch f", ch=8)
            ngr = (K - 1 + 7) // 8
            for g in range(ngr):
                klo, khi = 8 * g + 1, min(8 * g + 8, K - 1)
                apg = apgp.tile([128, 8, 8, 16], F32, tag="apg")
                for blk in range(8):
                    lo = t0s[blk] - (8 * g + 8)
                    ioff = max(0, -lo)
                    nc.sync.dma_start(
                        out=apg[blk * 16:(blk + 1) * 16]
                        .rearrange("p ch t cl -> p ch (t cl)")
                        [:, :, ioff * 16:128],
                        in_=av[:, :, (lo + ioff) * 16:(lo + 8) * 16])
                for k in range(klo, khi + 1):
                    i = 8 * (g + 1) - k
                    cand = pcand.tile([128, 128], F32, tag="cand")
                    # cand_psum[j, p] = trans[p, tag_j]; the alpha term is
                    # added by rowmax itself (Src0 + Src1), saving a matmul.
                    nc.tensor.matmul(cand[:], s_ohT[:], s_wt[:],
                                     start=True, stop=True)
                    nc.vector._custom_dve(
                        OPS["rowmax"],
                        out=s_cand[:].rearrange("p (ch cl) -> p ch cl", ch=8),
                        in0=apg[:, :, i, :],
                        in1=cand[:].rearrange("p (ch cl) -> p ch cl", ch=8),
                        s0=NEG_BIG, accum_out=s_mv[:])
                    if k < K - 1:
                        tail(k)
                    else:
                        nc.vector._custom_dve(
                            OPS["firstidx"], out=s_junk[:], in0=s_cand[:],
                            s0=s_mv[:], s1=1.0e9, accum_out=s_tag[:])
                        nc.scalar.copy(s_tags[:, k:k + 1], s_tag[:])

            nc.sync.dma_start(out=d_tagsraw[:], in_=s_tags[:])

    mybir.codegen_inst_isa_subclasses(nc)
    if legalize:
        legalize_waits(nc)
    return nc


_NC_CACHE = {}


def _get_nc(T):
    if T not in _NC_CACHE:
        _NC_CACHE[T] = build(T=T, UF=8, UB=8)
    return _NC_CACHE[T]


def _viterbi_row_np(pot_b, trans):
    """Exact numpy Viterbi decode for one batch row (fallback path)."""
    T, C = pot_b.shape
    alphas = np.empty((T, C), dtype=np.float32)
    alphas[0] = pot_b[0]
    for t in range(1, T):
        scores = alphas[t - 1][:, None] + trans
        alphas[t] = scores.max(axis=0).astype(np.float32) + pot_b[t]
    tags = np.empty(T, dtype=np.int64)
    tags[T - 1] = int(alphas[T - 1].argmax())
    for t in range(T - 1, 0, -1):
        cand = alphas[t - 1] + trans[:, tags[t]]
        tags[t - 1] = int(cand.argmax())
    return tags


def _assemble_core(raw, pot_core, trans, T):
    """raw [128, K] f32 -> tags [16, T] with exact coalescence check.

    Chain j = blk*16+b covers t in [blk*L, (blk+1)*L); its column k holds
    the tag at time t0(blk)-k.  Warmup columns k<W overlap block blk+1's
    coverage; if any position matches, the whole chain is exact."""
    L = (T + 7) // 8
    Wp = 32
    raw = raw.astype(np.int64)
    t0s = [min(blk * L + L - 1 + Wp, T - 1) for blk in range(8)]
    tags = np.empty((16, T), dtype=np.int64)
    for b in range(16):
        ok = True
        for blk in range(7, -1, -1):
            j = blk * 16 + b
            t0 = t0s[blk]
            ks = t0 - (blk * L + L - 1)          # first coverage column
            cov = raw[j, ks:ks + L][::-1]        # tags at t = blk*L .. +L-1
            if blk < 7:
                # warmup cols k in [0, ks): t = t0-k inside blk+1 coverage
                wt_ = raw[j, 0:ks][::-1]         # t = (blk+1)*L .. t0
                truth = tags[b, (blk + 1) * L:(blk + 1) * L + ks]
                if not np.any(wt_ == truth):
                    ok = False
            tags[b, blk * L:(blk + 1) * L] = cov
        if not ok:
            tags[b] = _viterbi_row_np(pot_core[b], trans)
    return tags


def kernel(inputs, transitions):
    """Full-input Viterbi decode on 8 NeuronCores (data-parallel over batch)."""
    from concourse import bass_utils

    inputs = np.asarray(inputs)
    transitions = np.asarray(transitions)
    B, T, C = inputs.shape
    n_cores = 8
    in_maps = host_prep(inputs, transitions, n_cores=n_cores)
    nc = _get_nc(T)
    res = bass_utils.run_bass_kernel_spmd(
        nc, in_maps, core_ids=list(range(n_cores)))
    bl = B // n_cores
    parts = []
    for core in range(n_cores):
        raw = res.results[core]["tagsraw"]
        pot_core = inputs[core * bl:(core + 1) * bl]
        parts.append(_assemble_core(raw, pot_core, transitions, T))
    tags = np.concatenate(parts, axis=0)  # [B, T]
    return tags.astype(inputs.dtype)



# revision 35
# speedup vs baseline: 1.1747x; 1.0031x over previous
"""Viterbi decode kernel for TRN2 (Bass/Tile) — custom-DVE fused version.

Layout (per core, B_loc=16 batch rows):
  partition p = b*8 + ch   (b in [0,16), ch in [0,8));  cur = ch*16 + cl
  TRW  [128, 16, 128] f32 : TRW[b*8+ch, cl, q] = trans[q, ch*16+cl]
  POT  [128, T*16]   f32 : POT[b*8+ch, t*16+cl]  = pot[b, t, ch*16+cl]
  AHIST DRAM [128, T*16] : alpha_t[b, cur] = AHIST[b*8+ch, t*16+cl]

Forward per t: ONE fused custom-DVE op (VIT_SEGMAX: running max of
  (TRW + alpha_bcast) with a per-cl-page reset via a hand-built
  SUB_DIM_DONE step state) -> page maxes at scr[:, :, 127]; stage =
  m + pot; 8x stream_shuffle -> ALPHA_P broadcast; AHIST DMA per group.

Backward per t (descending): onehot(tag) via iota is_eq; fp32 PE
  transpose; ONE fp32 selector matmul (exact: 0/1 weights) -> W column;
  fused VIT_ROWMAX (cand = alpha + wcol, accum max) -> m*; fused
  VIT_FIRSTIDX (first p with cand==m*, ties lowest) -> tag.
"""
from contextlib import ExitStack

import numpy as np

import concourse.bass as bass
import concourse.tile as tile
from concourse import mybir
from concourse import dve_spec as DS
from concourse import dve_ops as DO
from concourse.dve_spec import (
    Spec, Src0, Src1, C0, C1, Zero, MaxNeg, AluOp, scan, select, eq, Idx,
)
from concourse.dve_uop import DveOpSpec, Trigger

F32 = mybir.dt.float32
ADD = mybir.AluOpType.add
MAX = mybir.AluOpType.max
IS_EQ = mybir.AluOpType.is_equal
NEG_BIG = float(np.float32(-3.0e38))


# ---------------------------------------------------------------- custom ops
def _lower_segmax(spec, ver):
    """lower() with a hand-built FSM: seed -> steady <-> (SUB_DIM_DONE) step,
    where the step state re-seeds the scan accumulator with the current
    element (MAX(MaxNeg, expr)) so the fold restarts at each page."""
    DS._validate_body(spec, ver)
    spec2 = DS._hoist_stream_invariant_ops(spec)
    scans = DS._collect(spec2.body, DS.Scan)
    latches = DS._collect(spec2.body, DS.Latch)
    assert len(scans) == 1 and not latches
    n_lanes, n_stages = DS.N_LANES[ver], DS.N_STAGES[ver]
    p = DS._build_placement(spec2, scans, n_stages, n_lanes)
    seed_ov, _ = DS._scan_overrides(scans, p.node_stage)
    sc = scans[0]
    d = p.node_stage[sc]
    step_ov = {d: DS._Stage(sc.op, MaxNeg, sc.expr)}
    body_lvs = DS._body_scan_leaves(spec2)
    consume = (Src0 in body_lvs, Src1 in body_lvs)
    states = [
        DS._State(placement=p, overrides=seed_ov, trigger=DS.COUNT_ONCE,
                  repeat=1, next=(1, 0, 0), write_out=False),
        DS._State(placement=p, consume=consume,
                  trigger=(Trigger.SRC_TENSOR_DONE, Trigger.SUB_DIM_DONE,
                           Trigger.NONE),
                  next=(0, 2, 0)),
        DS._State(placement=p, consume=consume, overrides=step_ov,
                  trigger=(Trigger.SRC_TENSOR_DONE, Trigger.SUB_DIM_DONE,
                           Trigger.COUNT),
                  next=(0, 2, 1), repeat=1),
    ]
    uops = [DS._assemble(s) for s in states]
    for u in uops:
        u.validate(ver)
    return uops


def _register(op, uops_by_ver=None):
    if any(o.name == op.name for o in DO.OPS):
        return
    DO.OPS.append(op)
    DO.CUSTOM_DVE_SPECS[op.name] = op.spec
    row = DO._CUSTOM_DVE_ROW_BASE + len(DO.OPS) - 1
    assert row < 0x20
    DO._SUB_OPCODE_FOR_NAME[op.name] = row
    if uops_by_ver:
        for ver, uops in uops_by_ver.items():
            DO._COMPILE_CACHE[(op.name, ver)] = DveOpSpec(
                name=op.name, opcode=row, uops=uops,
                rd1_en=DS._has_src1(op.spec))


def _sha_for(spec, ver):
    s = DveOpSpec(name="tmp", opcode=1, uops=DS.lower(spec, ver=ver),
                  rd1_en=DS._has_src1(spec))
    return s.sha(ver)


_OPS_CACHE = {}


def get_ops():
    if _OPS_CACHE:
        return _OPS_CACHE
    ver = "v3"

    segmax_spec = Spec(
        body=scan(AluOp.MAX, Src0 + Src1),
        reference=lambda in0, in1, s0, s1, imm2: np.maximum.accumulate(
            (in0.astype(np.float32) + in1), axis=-1))
    segmax = DO.DveOp("VIT_SEGMAX", segmax_spec, subdim=True, uops_sha={})
    _register(segmax, {ver: _lower_segmax(segmax_spec, ver)})

    def _ref_rowmax(in0, in1, c0, c1, c2):
        b = (in0.astype(np.float32) + in1).astype(np.float32)
        m = np.maximum(c0, b.reshape(b.shape[0], -1).max(axis=-1, keepdims=True))
        return b, m

    rowmax_spec = Spec(body=Src0 + Src1, accum=AluOp.MAX, accum_init=C0,
                       reference=_ref_rowmax)
    rowmax = DO.DveOp("VIT_ROWMAX", rowmax_spec, subdim=False,
                      uops_sha={ver: None})
    rowmax.uops_sha[ver] = _sha_for(rowmax_spec, ver)
    _register(rowmax)

    def _ref_firstidx(in0, in1, c0, c1, c2):
        P = in0.shape[0]
        x = in0.reshape(P, -1)
        idx = np.broadcast_to(np.arange(x.shape[1], dtype=np.float32), x.shape)
        masked = np.where(x == c0, idx, c1)
        return masked, np.minimum(c1, masked.min(axis=-1, keepdims=True))

    firstidx_spec = Spec(body=select(eq(Src0, C0), Idx + Zero, C1),
                         accum=AluOp.MIN, accum_init=C1,
                         reference=_ref_firstidx)
    firstidx = DO.DveOp("VIT_FIRSTIDX", firstidx_spec, subdim=False,
                        uops_sha={ver: None})
    firstidx.uops_sha[ver] = _sha_for(firstidx_spec, ver)
    _register(firstidx)

    _OPS_CACHE.update(segmax=segmax, rowmax=rowmax, firstidx=firstidx)
    return _OPS_CACHE


# ------------------------------------------------------------------ utility
def legalize_waits(nc):
    """This container's walrus accepts at most ONE sync wait per
    instruction; Tile emits drains/noops with many.  Split them into
    single-wait NoOps on the same engine."""
    n_split = 0
    for f in nc.m.functions:
        for blk in f.blocks:
            new = []
            for inst in blk.instructions:
                si = inst.sync_info
                if si is not None and si.on_wait and len(si.on_wait) > 1:
                    waits = list(si.on_wait)
                    for j, w in enumerate(waits[:-1]):
                        new.append(mybir.InstNoOp(
                            name=f"{inst.name}-sw{j}", engine=inst.engine,
                            sync_info=mybir.SyncInfo(on_wait=[w], on_update=[])))
                        n_split += 1
                    inst.sync_info = mybir.SyncInfo(
                        on_wait=[waits[-1]], on_update=list(si.on_update))
                new.append(inst)
            blk.instructions = new
    return n_split


def host_prep(inputs_np, trans_np, n_cores=8):
    """Full inputs -> per-core input maps (list of dicts)."""
    B, T, C = inputs_np.shape
    assert C == 128 and B % n_cores == 0
    bl = B // n_cores  # 16

    transT = np.ascontiguousarray(trans_np.T).astype(np.float32)  # [c, q]

    # Rotated TRW for the log-doubling alpha broadcast: the alpha tile for
    # partition p=b*8+ch holds, at slot s (cols s*16..s*16+16), the stage of
    # channel (ch+s)&7, i.e. alpha[b, ((ch+s)&7)*16+cl2].  Bake the matching
    # q-permutation into TRW so segmax still sums alpha[q]+trans[q,c]:
    #   TRW[b*8+ch, cl, s*16+cl2] = trans[((ch+s)&7)*16+cl2, ch*16+cl]
    trw1 = np.empty((8, 16, 8, 16), dtype=np.float32)  # [ch, cl, s, cl2]
    for ch in range(8):
        for s in range(8):
            src = ((ch + s) & 7) * 16
            # trans[src+cl2, ch*16+cl] -> [cl, cl2]
            trw1[ch, :, s, :] = trans_np[src:src + 16, ch * 16:ch * 16 + 16].T
    trw = np.tile(trw1.reshape(8, 16 * 128)[None], (bl, 1, 1))
    trw = np.ascontiguousarray(trw.reshape(128, 16 * 128), dtype=np.float32)

    iota128 = np.ascontiguousarray(
        np.tile(np.arange(128, dtype=np.float32)[None, :], (128, 1)))
    id128 = np.eye(128, dtype=np.float32)

    in_maps = []
    for core in range(n_cores):
        pc = inputs_np[core * bl:(core + 1) * bl]  # [16, T, 128]
        pot = pc.reshape(bl, T, 8, 16).transpose(0, 2, 1, 3)
        pot = np.ascontiguousarray(pot.reshape(128, T * 16), dtype=np.float32)
        in_maps.append({
            "pot": pot, "trw": trw, "wt": transT,
            "iota128": iota128, "id128": id128,
        })
    return in_maps


def build(T=2048, UF=8, UB=8, legalize=True):
    """Build the Bass program. Returns nc."""
    OPS = get_ops()
    nc = bass.Bass()

    d_pot = nc.dram_tensor("pot", [128, T * 16], F32, kind="ExternalInput")
    d_trw = nc.dram_tensor("trw", [128, 16 * 128], F32, kind="ExternalInput")
    d_wt = nc.dram_tensor("wt", [128, 128], F32, kind="ExternalInput")
    d_id128 = nc.dram_tensor("id128", [128, 128], F32, kind="ExternalInput")
    d_iota128 = nc.dram_tensor(
        "iota128", [128, 128], F32, kind="ExternalInput")
    KK = (T + 7) // 8 + 24
    d_tagsraw = nc.dram_tensor("tagsraw", [128, KK], F32,
                               kind="ExternalOutput")
    d_ahist = nc.dram_tensor("ahist", [128, T * 16], F32, kind="Internal")
    ahist_bt = d_ahist.rearrange("(b ch) (t cl) -> b t ch cl", ch=8, cl=16)

    with tile.TileContext(nc) as tc, ExitStack() as ctx:
        singles = ctx.enter_context(tc.tile_pool(name="singles", bufs=1))

        # ---------------- forward ----------------
        with (
            tc.tile_pool(name="potp", bufs=1) as potp,
            tc.tile_pool(name="stp", bufs=2) as stp,
            tc.tile_pool(name="scrp", bufs=2) as scrp,
        ):
            s_trw = singles.tile([128, 16, 128], F32)
            s_alpha = singles.tile([128, 128], F32)  # ALPHA_P
            nc.sync.dma_start(
                out=s_trw[:], in_=d_trw.rearrange("p (cl q) -> p cl q", cl=16))

            s_pot = potp.tile([128, T * 16], F32)
            NPC = 8  # split preload so early compute can start sooner
            for c in range(NPC):
                sl = slice(c * T * 16 // NPC, (c + 1) * T * 16 // NPC)
                nc.sync.dma_start(out=s_pot[:, sl], in_=d_pot[:, sl])

            # touch each preload chunk on DVE so the loop body carries no
            # extra DMA-queue waits (back-edge drain has limited wait slots)
            s_touch = singles.tile([128, NPC], F32)
            for c in range(NPC):
                nc.vector.tensor_copy(
                    s_touch[:, c:c + 1],
                    s_pot[:, c * T * 16 // NPC:c * T * 16 // NPC + 1])

            # alpha broadcast: slot s of partition (b,ch) holds the stage of
            # channel (ch+s)&7; TRW is pre-rotated to match.  The stage is
            # written straight into slot 0 by the producer, so only slots
            # 1-3 (independent -> pipeline) plus one rot-4 shuffle remain.
            def bcast_alpha():
                for s in (1, 2, 3):
                    mask = [(j & ~7) | (((j & 7) + s) & 7) for j in range(32)]
                    nc.vector.stream_shuffle(
                        s_alpha[:, s * 16:(s + 1) * 16], s_alpha[:, 0:16],
                        mask)
                mask4 = [(j & ~7) | (((j & 7) + 4) & 7) for j in range(32)]
                nc.vector.stream_shuffle(
                    s_alpha[:, 64:128], s_alpha[:, 0:64], mask4)

            # t=0 init
            st0 = stp.tile([128, UF * 16], F32, tag="stage")
            nc.vector.tensor_copy(s_alpha[:, 0:16], s_pot[:, 0:16])
            nc.scalar.copy(st0[:, 0:16], s_alpha[:, 0:16])
            bcast_alpha()
            nc.sync.dma_start(out=d_ahist[:, 0:16], in_=st0[:, 0:16])

            def fwd_group(iv0, unroll):
                stage = stp.tile([128, UF * 16], F32, tag="stage")
                for k in range(unroll):
                    iv = iv0 + k * 16
                    scr = scrp.tile([128, 16, 128], F32, tag="scr")
                    alb = s_alpha[:]
                    al_bcast = bass.AP(
                        tensor=alb.tensor, offset=alb.offset,
                        ap=[list(alb.ap[0]), [0, 16], [1, 128]])
                    nc.vector._custom_dve(
                        OPS["segmax"], out=scr[:], in0=s_trw[:], in1=al_bcast)
                    # page maxes at scr[:, :, 127]
                    so = scr[:]
                    m_ap = bass.AP(tensor=so.tensor, offset=so.offset + 127,
                                   ap=[list(so.ap[0]), [128, 16]])
                    ksl = slice(k * 16, (k + 1) * 16)
                    # write the new stage straight into broadcast slot 0;
                    # the idle Scalar engine copies it to the AHIST staging
                    # tile off the DVE critical stream.
                    nc.vector.tensor_add(
                        s_alpha[:, 0:16], m_ap, s_pot[:, iv:iv + 16])
                    nc.scalar.copy(stage[:, ksl], s_alpha[:, 0:16])
                    bcast_alpha()
                nc.sync.dma_start(
                    out=d_ahist[:, iv0:iv0 + unroll * 16],
                    in_=stage[:, 0:unroll * 16])

            ngrp, rem = divmod(T - 1, UF)
            for g in range(ngrp):
                fwd_group(16 + g * UF * 16, UF)
            if rem:
                fwd_group(16 + ngrp * UF * 16, rem)

        # ---------------- backward: 128 parallel chains ----------------
        # Chain j = blk*16 + b backtraces time-block blk (length L=256)
        # plus a W=64 warmup into block blk+1 (greedy-started; coalescence
        # verified exactly on host, with numpy fallback).  All 128 chains
        # advance in lockstep: K=L+W steps instead of T serial steps.
        L, W = (T + 7) // 8, 24
        K = L + W
        t0s = [min(blk * L + L - 1 + W, T - 1) for blk in range(8)]
        with (
            tc.tile_pool(name="apg", bufs=3) as apgp,
            tc.tile_pool(name="pcand", bufs=2, space="PSUM") as pcand,
            tc.tile_pool(name="ptr", bufs=2, space="PSUM") as ptr,
        ):
            s_wt = singles.tile([128, 128], F32)
            nc.sync.dma_start(out=s_wt[:], in_=d_wt[:])
            s_id128 = singles.tile([128, 128], F32)
            nc.sync.dma_start(out=s_id128[:], in_=d_id128[:])
            s_iota128 = singles.tile([128, 128], F32)
            nc.sync.dma_start(out=s_iota128[:], in_=d_iota128[:])

            BF16 = mybir.dt.bfloat16
            s_tags = singles.tile([128, K], F32)
            s_tag = singles.tile([128, 1], F32)
            s_mv = singles.tile([128, 1], F32)
            # one-hot + identity are exact in bf16 -> single-pass transpose
            s_oh = singles.tile([128, 128], BF16)
            s_idb = singles.tile([128, 128], BF16)
            nc.vector.tensor_copy(s_idb[:], s_id128[:])
            s_ohT = singles.tile([128, 128], F32)
            s_cand = singles.tile([128, 128], F32)
            s_junk = singles.tile([128, 128], F32)
            s_zero = singles.tile([128, 128], F32)
            nc.vector.memset(s_zero[:], 0)

            def tail(k):
                # from cand stream in s_cand + row max in s_mv: pick first
                # argmax as tag, record it, and build one-hot^T for the next
                # gather matmul.
                nc.vector._custom_dve(
                    OPS["firstidx"], out=s_junk[:], in0=s_cand[:],
                    s0=s_mv[:], s1=1.0e9, accum_out=s_tag[:])
                nc.scalar.copy(s_tags[:, k:k + 1], s_tag[:])
                nc.vector.tensor_scalar(
                    out=s_oh[:], in0=s_iota128[:], scalar1=s_tag[:],
                    scalar2=None, op0=IS_EQ)
                pT = ptr.tile([128, 128], BF16, tag="pT")
                nc.tensor.transpose(pT[:], s_oh[:], s_idb[:])
                nc.vector.tensor_copy(s_ohT[:], pT[:])

            # k=0: greedy init tag[j] = argmax_c alpha_{t0(blk)}[b, c]
            s_a0 = singles.tile([128, 128], F32)
            for blk in range(8):
                nc.sync.dma_start(
                    out=s_a0[blk * 16:(blk + 1) * 16, :],
                    in_=ahist_bt[:, t0s[blk]:t0s[blk] + 1, :, :])
            nc.vector._custom_dve(
                OPS["rowmax"], out=s_cand[:], in0=s_a0[:], in1=s_zero[:],
                s0=NEG_BIG, accum_out=s_mv[:])
            tail(0)

            # lockstep groups of 8: prefetch alpha rows for all 8 blocks.
            # apg layout [j, ch, i, cl]: the (i, cl) block of one (b, ch)
            # DRAM row is contiguous, keeping both DMA APs at 3 dims.
            av = d_ahist.rearrange("(b ch) f -> b ch f", ch=8)
            ngr = (K - 1 + 7) // 8
            for g in range(ngr):
                klo, khi = 8 * g + 1, min(8 * g + 8, K - 1)
                apg = apgp.tile([128, 8, 8, 16], F32, tag="apg")
                for blk in range(8):
                    lo = t0s[blk] - (8 * g + 8)
                    ioff = max(0, -lo)
                    nc.sync.dma_start(
                        out=apg[blk * 16:(blk + 1) * 16]
                        .rearrange("p ch t cl -> p ch (t cl)")
                        [:, :, ioff * 16:128],
                        in_=av[:, :, (lo + ioff) * 16:(lo + 8) * 16])
                for k in range(klo, khi + 1):
                    i = 8 * (g + 1) - k
                    cand = pcand.tile([128, 128], F32, tag="cand")
                    # cand_psum[j, p] = trans[p, tag_j]; the alpha term is
                    # added by rowmax itself (Src0 + Src1), saving a matmul.
                    nc.tensor.matmul(cand[:], s_ohT[:], s_wt[:],
                                     start=True, stop=True)
                    nc.vector._custom_dve(
                        OPS["rowmax"],
                        out=s_cand[:].rearrange("p (ch cl) -> p ch cl", ch=8),
                        in0=apg[:, :, i, :],
                        in1=cand[:].rearrange("p (ch cl) -> p ch cl", ch=8),
                        s0=NEG_BIG, accum_out=s_mv[:])
                    if k < K - 1:
                        tail(k)
                    else:
                        nc.vector._custom_dve(
                            OPS["firstidx"], out=s_junk[:], in0=s_cand[:],
                            s0=s_mv[:], s1=1.0e9, accum_out=s_tag[:])
                        nc.scalar.copy(s_tags[:, k:k + 1], s_tag[:])

            nc.sync.dma_start(out=d_tagsraw[:], in_=s_tags[:])

    mybir.codegen_inst_isa_subclasses(nc)
    if legalize:
        legalize_waits(nc)
    return nc


_NC_CACHE = {}


def _get_nc(T):
    if T not in _NC_CACHE:
        _NC_CACHE[T] = build(T=T, UF=8, UB=8)
    return _NC_CACHE[T]


def _viterbi_row_np(pot_b, trans):
    """Exact numpy Viterbi decode for one batch row (fallback path)."""
    T, C = pot_b.shape
    alphas = np.empty((T, C), dtype=np.float32)
    alphas[0] = pot_b[0]
    for t in range(1, T):
        scores = alphas[t - 1][:, None] + trans
        alphas[t] = scores.max(axis=0).astype(np.float32) + pot_b[t]
    tags = np.empty(T, dtype=np.int64)
    tags[T - 1] = int(alphas[T - 1].argmax())
    for t in range(T - 1, 0, -1):
        cand = alphas[t - 1] + trans[:, tags[t]]
        tags[t - 1] = int(cand.argmax())
    return tags


def _assemble_core(raw, pot_core, trans, T):
    """raw [128, K] f32 -> tags [16, T] with exact coalescence check.

    Chain j = blk*16+b covers t in [blk*L, (blk+1)*L); its column k holds
    the tag at time t0(blk)-k.  Warmup columns k<W overlap block blk+1's
    coverage; if any position matches, the whole chain is exact."""
    L = (T + 7) // 8
    Wp = 24
    raw = raw.astype(np.int64)
    t0s = [min(blk * L + L - 1 + Wp, T - 1) for blk in range(8)]
    tags = np.empty((16, T), dtype=np.int64)
    for b in range(16):
        ok = True
        for blk in range(7, -1, -1):
            j = blk * 16 + b
            t0 = t0s[blk]
            ks = t0 - (blk * L + L - 1)          # first coverage column
            cov = raw[j, ks:ks + L][::-1]        # tags at t = blk*L .. +L-1
            if blk < 7:
                # warmup cols k in [0, ks): t = t0-k inside blk+1 coverage
                wt_ = raw[j, 0:ks][::-1]         # t = (blk+1)*L .. t0
                truth = tags[b, (blk + 1) * L:(blk + 1) * L + ks]
                if not np.any(wt_ == truth):
                    ok = False
            tags[b, blk * L:(blk + 1) * L] = cov
        if not ok:
            tags[b] = _viterbi_row_np(pot_core[b], trans)
    return tags


def kernel(inputs, transitions):
    """Full-input Viterbi decode on 8 NeuronCores (data-parallel over batch)."""
    from concourse import bass_utils

    inputs = np.asarray(inputs)
    transitions = np.asarray(transitions)
    B, T, C = inputs.shape
    n_cores = 8
    in_maps = host_prep(inputs, transitions, n_cores=n_cores)
    nc = _get_nc(T)
    res = bass_utils.run_bass_kernel_spmd(
        nc, in_maps, core_ids=list(range(n_cores)))
    bl = B // n_cores
    parts = []
    for core in range(n_cores):
        raw = res.results[core]["tagsraw"]
        pot_core = inputs[core * bl:(core + 1) * bl]
        parts.append(_assemble_core(raw, pot_core, transitions, T))
    tags = np.concatenate(parts, axis=0)  # [B, T]
    return tags.astype(inputs.dtype)

